# revision 4
# baseline (speedup 1.0000x reference)
"""ChebNet (K=3, L=2) forward on 8 Trainium2 NeuronCores.

Node-sharded SPMD: each core owns 6250 dst rows. Chebyshev recurrence in the
U-basis (U1 = A_hat U0, U2 = A_hat U1; out = U0(W0-W2) - U1 W1 + 2 U2 W2).
SpMM = dma_gather of per-edge feature rows from a replicated DRAM table (two
int16-addressable halves) + PE mask-matmuls with on-device-generated masks
carrying norm[src]*norm[dst] edge weights. Fixed windows of 32 dst rows,
6 x 128 edge slots per window (3 per table half), 8 windows per PSUM chunk.
Full node tables are rebuilt between SpMMs with AllGather (3 per pass).
"""
import os
import numpy as np

N, E, D, K, L = 50000, 800000, 64, 3, 2
NCORES = 8
NSH = N // NCORES              # 6250 dst rows per core
WIN = 32                       # dst rows per window
NWIN = 208                     # windows per core -> 6656 padded cols
NCOLS = NWIN * WIN
CWIN = 8                       # windows per PSUM chunk
NCH = NWIN // CWIN             # 26 chunks
CCOLS = CWIN * WIN             # 256 cols per chunk
TPW = 3                        # tiles per window per table half
ROWS_PAD = NCOLS               # padded rows per core block in the table
TBL_ROWS = NCORES * ROWS_PAD   # 53248
HALF = TBL_ROWS // 2           # 26624 (< 32768: int16-reachable)
NTILE_CH = CWIN * 2 * TPW      # 48 mask tiles per chunk
SLOTS_CH = CWIN * 2 * TPW * 128  # 6144 slots per chunk
SLOTS_TOT = NCH * SLOTS_CH     # 159744
NTILE_TOT = SLOTS_TOT // 128   # 1248
GIDX = 1024                    # rows per dma_gather call
GPC = SLOTS_CH // 2 // GIDX    # 3 gathers per half per chunk

_CACHE = {}


# ---------------------------------------------------------------------------
# Workaround for this walrus build: any instruction carrying >1 sync wait is
# rejected ("Too many sync wait commands"). Hoist extras onto 1-wait NoOps on
# the same engine (per-engine program order preserves semantics).
_ws_counter = [0]


def _split_multiwaits(nc):
    import concourse.mybir as mybir
    n_split = 0
    for fn in nc.m.functions:
        for bb in fn.blocks:
            new_list = []
            changed = False
            for inst in bb.instructions:
                si = inst.sync_info
                waits = list(si.on_wait) if si is not None else []
                if len(waits) > 1:
                    changed = True
                    for w in waits[:-1]:
                        _ws_counter[0] += 1
                        nop = mybir.InstNoOp(
                            name=f"waitsplit-{_ws_counter[0]}",
                            ins=[], outs=[],
                            sync_info=mybir.SyncInfo(on_wait=[w], on_update=[]),
                        )
                        nop.engine = inst.engine
                        nc.register_instruction(nop, overwrite=True)
                        new_list.append(nop)
                        n_split += 1
                    si.on_wait = waits[-1:]
                new_list.append(inst)
            if changed:
                bb.instructions[:] = new_list
    return n_split


def _finalize_with_split(nc):
    import concourse.bass as _bass
    nc.compile()           # Bacc passes (incl. library-load insertion)
    _split_multiwaits(nc)  # after replace_nops_with_events, before freeze
    _bass.Bass.finalize(nc)


def _build_runner(nc, n_cores):
    """SPMD runner over the axon PJRT backend (keeps the jitted executable
    and device-resident inputs so repeat calls can be timed)."""
    import jax
    from jax.sharding import Mesh, PartitionSpec
    from jax.experimental.shard_map import shard_map
    import concourse.mybir as mybir
    from concourse.bass2jax import (
        _bass_exec_p, install_neuronx_cc_hook, partition_id_tensor)

    install_neuronx_cc_hook()
    partition_name = nc.partition_id_tensor.name if nc.partition_id_tensor else None

    in_names, out_names, out_avals, zero_outs = [], [], [], []
    for alloc in nc.m.functions[0].allocations:
        if not isinstance(alloc, mybir.MemoryLocationSet):
            continue
        name = alloc.memorylocations[0].name
        if alloc.kind == "ExternalInput":
            if name != partition_name:
                in_names.append(name)
        elif alloc.kind == "ExternalOutput":
            shape = tuple(alloc.tensor_shape)
            dtype = mybir.dt.np(alloc.dtype)
            out_names.append(name)
            out_avals.append(jax.core.ShapedArray(shape, dtype))
            zero_outs.append(np.zeros(shape, dtype))
    n_params = len(in_names)
    all_in_names = list(in_names) + list(out_names)
    if partition_name is not None:
        all_in_names.append(partition_name)

    def _body(*args):
        operands = list(args)
        if partition_name is not None:
            operands.append(partition_id_tensor())
        outs = _bass_exec_p.bind(
            *operands,
            out_avals=tuple(out_avals),
            in_names=tuple(all_in_names),
            out_names=tuple(out_names),
            lowering_input_output_aliases=(),
            sim_require_finite=True,
            sim_require_nnan=True,
            nc=nc,
        )
        return tuple(outs)

    devices = jax.devices()[:n_cores]
    mesh = Mesh(np.asarray(devices), ("core",))
    in_specs = (PartitionSpec("core"),) * (n_params + len(out_names))
    out_specs = (PartitionSpec("core"),) * len(out_names)
    sharded = jax.jit(
        shard_map(_body, mesh=mesh, in_specs=in_specs, out_specs=out_specs,
                  check_rep=False),
        keep_unused=True,
    )

    def run(in_maps, iters=1):
        import time as _time
        per_core = [[np.asarray(m[name]) for name in in_names] for m in in_maps]
        concat_in = [
            np.concatenate([per_core[c][i] for c in range(n_cores)], axis=0)
            for i in range(n_params)
        ]
        concat_zeros = [
            np.zeros((n_cores * z.shape[0], *z.shape[1:]), z.dtype)
            for z in zero_outs
        ]
        sharding = jax.sharding.NamedSharding(mesh, PartitionSpec("core"))
        dev_in = [jax.device_put(a, sharding) for a in concat_in + concat_zeros]
        out = sharded(*dev_in)
        jax.block_until_ready(out)
        times = []
        for _ in range(iters):
            t0 = _time.perf_counter()
            out = sharded(*dev_in)
            jax.block_until_ready(out)
            times.append(_time.perf_counter() - t0)
        results = [
            {name: np.asarray(out[i]).reshape(n_cores, *out_avals[i].shape)[c]
             for i, name in enumerate(out_names)}
            for c in range(n_cores)
        ]
        return results, times

    return run


def _host_prep(features, src, dst, W, b, pw, pb):
    src = np.asarray(src).astype(np.int64)
    dst = np.asarray(dst).astype(np.int64)
    features = np.asarray(features, dtype=np.float32)
    W = np.asarray(W, dtype=np.float32)
    b = np.asarray(b, dtype=np.float32)
    pw = np.asarray(pw, dtype=np.float32).reshape(D, 1)
    pb = np.asarray(pb, dtype=np.float32).reshape(1)

    deg = np.bincount(dst, minlength=N).astype(np.float32)
    norm = np.clip(deg, 1.0, None) ** -0.5
    wedge = (norm[src] * norm[dst]).astype(np.float32)

    Wflat = np.zeros((D, L * 3 * D), dtype=np.float32)
    for l in range(L):
        for t, Wt in enumerate((W[l, 0] - W[l, 2], -W[l, 1], 2.0 * W[l, 2])):
            Wflat[:, (l * 3 + t) * D:(l * 3 + t + 1) * D] = Wt

    feat_pad = np.zeros((TBL_ROWS, D), dtype=np.float32)
    for j in range(NCORES):
        feat_pad[ROWS_PAD * j:ROWS_PAD * j + NSH] = features[NSH * j:NSH * (j + 1)]

    src_pad = (src // NSH) * ROWS_PAD + (src % NSH)
    core_of = dst // NSH
    dl_all = dst - core_of * NSH

    in_maps = []
    for i in range(NCORES):
        sel = core_of == i
        e_src = src_pad[sel]
        e_dl = dl_all[sel]
        e_w = wedge[sel]
        win = e_dl // WIN
        col = (e_dl % WIN).astype(np.float32)
        half = (e_src >= HALF).astype(np.int64)

        # order edges by (window, half); compute slot within (window, half)
        key = win * 2 + half
        order = np.argsort(key, kind="stable")
        ks = key[order]
        # rank within group
        grp_start = np.searchsorted(ks, np.arange(NWIN * 2), side="left")
        grp_cnt = np.diff(np.append(grp_start, ks.size))
        if grp_cnt.max() > TPW * 128:
            raise RuntimeError(f"window overflow core {i}: {grp_cnt.max()}")
        rank = np.arange(ks.size) - grp_start[ks]

        ww = win[order]
        hh = half[order]
        c_ = ww // CWIN
        wl = ww % CWIN
        slot = (c_ * SLOTS_CH + hh * (SLOTS_CH // 2) + wl * (TPW * 128) + rank)

        idx_slots = np.zeros(SLOTS_TOT, dtype=np.int16)
        col_slots = np.zeros(SLOTS_TOT, dtype=np.float32)
        w_slots = np.zeros(SLOTS_TOT, dtype=np.float32)
        idx_slots[slot] = (e_src[order] - hh * HALF).astype(np.int16)
        col_slots_tmp = col[order]
        w_slots_tmp = e_w[order]

        # mask tile layout: tile index mt within chunk = wl*(2*TPW) + hh*TPW + rank//128
        tj = rank // 128
        part = rank % 128
        mt = c_ * NTILE_CH + wl * (2 * TPW) + hh * TPW + tj
        col_slots[mt * 128 + part] = col_slots_tmp
        w_slots[mt * 128 + part] = w_slots_tmp

        # gather index wrap: gather g slot s -> idx_arr[s%16, g*64 + s//16]
        wrapped = idx_slots.reshape(SLOTS_TOT // GIDX, 64, 16).transpose(0, 2, 1)
        idx_arr16 = wrapped.reshape(SLOTS_TOT // GIDX, 16, 64)
        idx_arr = np.zeros((16, SLOTS_TOT // 16), dtype=np.int16)
        for g in range(SLOTS_TOT // GIDX):
            idx_arr[:, g * 64:(g + 1) * 64] = idx_arr16[g]
        idx_arr = np.tile(idx_arr, (8, 1))

        dcol = col_slots.reshape(NTILE_TOT, 128).T.copy()
        wval = w_slots.reshape(NTILE_TOT, 128).T.copy()

        f0T = np.zeros((D, NCOLS), dtype=np.float32)
        f0T[:, :NSH] = features[NSH * i:NSH * (i + 1)].T

        iota = np.tile(np.arange(WIN, dtype=np.float32)[None, :], (128, 1))

        in_maps.append({
            "feat_pad": feat_pad,
            "f0T": f0T,
            "idx_all": idx_arr,
            "dcol": dcol,
            "wval": wval,
            "iota": iota,
            "Wflat": Wflat,
            "bvec": b.T.copy(),
            "pwv": pw,
            "pbv": pb.reshape(1, 1),
        })
    return in_maps


def _build_nc():
    import concourse.bacc as bacc
    import concourse.mybir as mybir
    import concourse.tile as tile
    from concourse.masks import make_identity
    f32 = mybir.dt.float32

    nc = bacc.Bacc("TRN2", num_swdge_queues=4)
    feat_pad = nc.declare_dram_parameter("feat_pad", [TBL_ROWS, D], f32, isOutput=False)
    f0T_in = nc.declare_dram_parameter("f0T", [D, NCOLS], f32, isOutput=False)
    idx_in = nc.declare_dram_parameter("idx_all", [128, SLOTS_TOT // 16], mybir.dt.int16, isOutput=False)
    dcol_in = nc.declare_dram_parameter("dcol", [128, NTILE_TOT], f32, isOutput=False)
    wval_in = nc.declare_dram_parameter("wval", [128, NTILE_TOT], f32, isOutput=False)
    iota_in = nc.declare_dram_parameter("iota", [128, WIN], f32, isOutput=False)
    W_in = nc.declare_dram_parameter("Wflat", [D, L * 3 * D], f32, isOutput=False)
    b_in = nc.declare_dram_parameter("bvec", [D, L], f32, isOutput=False)
    pw_in = nc.declare_dram_parameter("pwv", [D, 1], f32, isOutput=False)
    pb_in = nc.declare_dram_parameter("pbv", [1, 1], f32, isOutput=False)
    y_out = nc.declare_dram_parameter("y", [NSH, 1], f32, isOutput=True)

    ag = {}
    for nm in ("u1", "h1", "u1b"):
        ag[nm] = (
            nc.dram_tensor(f"agin_{nm}", [ROWS_PAD, D], f32),
            nc.dram_tensor(f"agout_{nm}", [TBL_ROWS, D], f32, addr_space="Shared"),
        )

    with tile.TileContext(nc) as tc:
        with (
            tc.tile_pool(name="const", bufs=1) as cp,
            tc.tile_pool(name="idxp", bufs=2) as ip,
            tc.tile_pool(name="gbuf", bufs=2) as gp,
            tc.tile_pool(name="mbuf", bufs=2) as mp,
            tc.tile_pool(name="sT", bufs=1) as sp,
            tc.tile_pool(name="rows", bufs=1) as rp,
            tc.tile_pool(name="spsum", bufs=2, space="PSUM") as pp,
            tc.tile_pool(name="opsum", bufs=2, space="PSUM") as tp,
        ):
            dcol = cp.tile([128, NTILE_TOT], f32)
            nc.sync.dma_start(out=dcol[:], in_=dcol_in[:])
            wval = cp.tile([128, NTILE_TOT], f32)
            nc.sync.dma_start(out=wval[:], in_=wval_in[:])
            iota = cp.tile([128, WIN], f32)
            nc.sync.dma_start(out=iota[:], in_=iota_in[:])
            wfl = cp.tile([D, L * 3 * D], f32)
            nc.sync.dma_start(out=wfl[:], in_=W_in[:])
            bv = cp.tile([D, L], f32)
            nc.sync.dma_start(out=bv[:], in_=b_in[:])
            pwv = cp.tile([D, 1], f32)
            nc.sync.dma_start(out=pwv[:], in_=pw_in[:])
            pbv = cp.tile([1, 1], f32)
            nc.sync.dma_start(out=pbv[:], in_=pb_in[:])
            f0T = cp.tile([D, NCOLS], f32)
            nc.sync.dma_start(out=f0T[:], in_=f0T_in[:])
            ident = cp.tile([128, 128], f32)
            make_identity(nc, ident[:])

            u1T = sp.tile([D, NCOLS], f32, tag="u1T")
            h1T = sp.tile([D, NCOLS], f32, tag="h1T")
            yrow = sp.tile([1, NCOLS], f32, tag="yrow")

            gq = [0]

            def spmm_chunk(table, c, tag):
                """Gathers + mask gen + PE reduce for chunk c. Returns psum
                tile [64, CCOLS] (caller evacuates / consumes)."""
                idxc = ip.tile([128, SLOTS_CH // 16], mybir.dt.int16, tag="idxc",
                               name=f"idxc_{tag}_{c}")
                nc.sync.dma_start(
                    out=idxc[:],
                    in_=idx_in[:, c * (SLOTS_CH // 16):(c + 1) * (SLOTS_CH // 16)])
                glo = gp.tile([128, CWIN * TPW, D], f32, tag="glo", name=f"glo_{tag}_{c}")
                ghi = gp.tile([128, CWIN * TPW, D], f32, tag="ghi", name=f"ghi_{tag}_{c}")
                for h, gbuf in ((0, glo), (1, ghi)):
                    tab = table[h * HALF:(h + 1) * HALF, :]
                    for g in range(GPC):
                        off = (h * (SLOTS_CH // 2) + g * GIDX) // 16
                        nc.gpsimd.dma_gather(
                            gbuf[:, g * (GIDX // 128):(g + 1) * (GIDX // 128), :],
                            tab,
                            idxc[:, off:off + GIDX // 16],
                            GIDX, GIDX, D,
                            queue_num=gq[0] % 4,
                        )
                        gq[0] += 1
                mask = mp.tile([128, NTILE_CH * WIN], f32, tag="mask",
                               name=f"mask_{tag}_{c}")
                m3 = mask[:].rearrange("p (t o) -> p t o", o=WIN)
                i3 = iota[:].rearrange("p (o t) -> p o t", o=1).to_broadcast(
                    [128, NTILE_CH, WIN])
                d3 = dcol[:, c * NTILE_CH:(c + 1) * NTILE_CH].rearrange(
                    "p (t o) -> p t o", o=1).to_broadcast([128, NTILE_CH, WIN])
                w3 = wval[:, c * NTILE_CH:(c + 1) * NTILE_CH].rearrange(
                    "p (t o) -> p t o", o=1).to_broadcast([128, NTILE_CH, WIN])
                nc.vector.tensor_tensor(out=m3, in0=i3, in1=d3,
                                        op=mybir.AluOpType.is_equal)
                nc.vector.tensor_tensor(out=m3, in0=m3, in1=w3,
                                        op=mybir.AluOpType.mult)
                ps = pp.tile([64, CCOLS], f32, tag="spsum", name=f"ps_{tag}_{c}")
                for w in range(CWIN):
                    for j in range(2 * TPW):
                        lhsT = (glo if j < TPW else ghi)[:, TPW * w + (j % TPW), :]
                        mt = w * (2 * TPW) + j
                        nc.tensor.matmul(
                            ps[:, WIN * w:WIN * (w + 1)],
                            lhsT,
                            mask[:, mt * WIN:(mt + 1) * WIN],
                            start=(j == 0), stop=(j == 2 * TPW - 1),
                        )
                return ps

            def spmm(table, out_sT, tag):
                for c in range(NCH):
                    ps = spmm_chunk(table, c, tag)
                    nc.vector.tensor_copy(
                        out=out_sT[:, c * CCOLS:(c + 1) * CCOLS], in_=ps[:])

            def spmm_fused_dense(table, l, u0T, u1T_, outT, tag):
                """SpMM for U2 fused with the dense layer + (layer 2) head."""
                for c in range(NCH):
                    ps = spmm_chunk(table, c, tag)
                    u2c = mp.tile([64, CCOLS], f32, tag="u2c", name=f"u2c_{tag}_{c}")
                    nc.vector.tensor_copy(out=u2c[:], in_=ps[:])
                    dp = tp.tile([64, CCOLS], f32, tag="dpsum", name=f"dp_{tag}_{c}")
                    for t, uT in enumerate((u0T, u1T_, u2c)):
                        rhs = uT[:] if t == 2 else uT[:, c * CCOLS:(c + 1) * CCOLS]
                        nc.tensor.matmul(
                            dp[:],
                            wfl[:, (l * 3 + t) * D:(l * 3 + t + 1) * D],
                            rhs,
                            start=(t == 0), stop=(t == 2),
                        )
                    nc.scalar.activation(
                        out=outT[:, c * CCOLS:(c + 1) * CCOLS],
                        in_=dp[:],
                        func=mybir.ActivationFunctionType.Relu,
                        bias=bv[:, l:l + 1],
                        scale=1.0,
                    )
                    if l == L - 1:
                        hp = tp.tile([1, CCOLS], f32, tag="hpsum", name=f"hp_{c}")
                        nc.tensor.matmul(
                            hp[:], pwv[:],
                            outT[:, c * CCOLS:(c + 1) * CCOLS],
                            start=True, stop=True)
                        nc.vector.tensor_scalar(
                            out=yrow[:1, c * CCOLS:(c + 1) * CCOLS],
                            in0=hp[:], scalar1=pbv[:1, :1], scalar2=None,
                            op0=mybir.AluOpType.add)

            def transpose_out(sT, agin, tag):
                rows = rp.tile([128, NCOLS // 128, D], f32, tag="rows",
                               name=f"rows_{tag}")
                for k in range(NCOLS // 128):
                    tps = tp.tile([128, D], f32, tag="tpsum", name=f"tps_{tag}_{k}")
                    nc.tensor.transpose(tps[:], sT[:, k * 128:(k + 1) * 128],
                                        ident[:64, :64])
                    nc.vector.tensor_copy(out=rows[:, k, :], in_=tps[:])
                nc.sync.dma_start(
                    out=agin.ap().rearrange("(k p) d -> p k d", p=128), in_=rows[:])

            def allgather(nm):
                agin, agout = ag[nm]
                nc.gpsimd.collective_compute(
                    "AllGather",
                    mybir.AluOpType.bypass,
                    ins=[agin.ap().opt()],
                    outs=[agout.ap().opt()],
                    replica_groups=[list(range(NCORES))],
                )

            # ---- layer 1 ----
            spmm(feat_pad, u1T, "s1")
            transpose_out(u1T, ag["u1"][0], "t1")
            allgather("u1")
            # h1T <- relu(f0 Wa + u1 Wb + u2 Wc + b0), u2 fused from SpMM2
            spmm_fused_dense(ag["u1"][1], 0, f0T, u1T, h1T, "s2")
            transpose_out(h1T, ag["h1"][0], "t2")
            allgather("h1")
            # ---- layer 2 ----
            spmm(ag["h1"][1], u1T, "s3")
            transpose_out(u1T, ag["u1b"][0], "t3")
            allgather("u1b")
            # h2T chunks fused; head writes yrow (h2 stored into... reuse h1T? no:
            # dense2 reads h1T as u0 while writing outT chunks -> use separate)
            h2T = sp.tile([D, NCOLS], f32, tag="h2T")
            spmm_fused_dense(ag["u1b"][1], 1, h1T, u1T, h2T, "s4")
            nc.sync.dma_start(out=y_out[:, :], in_=yrow[:1, :NSH])

    _finalize_with_split(nc)
    return nc


def _get_runner():
    if "runner" in _CACHE:
        return _CACHE["runner"]
    nc = _build_nc()
    _CACHE["runner"] = _build_runner(nc, NCORES)
    return _CACHE["runner"]


def kernel(features, src, dst, W, b, pw, pb):
    in_maps = _host_prep(features, src, dst, W, b, pw, pb)
    run = _get_runner()
    results, times = run(in_maps, iters=1)
    _CACHE["last_times"] = times
    y = np.concatenate([results[i]["y"] for i in range(NCORES)], axis=0)
    return y.astype(np.float32)


# revision 6
# speedup vs baseline: 20.5933x; 20.5933x over previous
"""ChebNet (K=3, L=2) forward on 8 Trainium2 NeuronCores.

Node-sharded SPMD: each core owns 6250 dst rows. Chebyshev recurrence in the
U-basis (U1 = A_hat U0, U2 = A_hat U1; out = U0(W0-W2) - U1 W1 + 2 U2 W2).
SpMM = dma_gather of per-edge feature rows from a replicated DRAM table (two
int16-addressable halves) + PE mask-matmuls with on-device-generated masks
carrying norm[src]*norm[dst] edge weights. Fixed windows of 32 dst rows,
6 x 128 edge slots per window (3 per table half), 8 windows per PSUM chunk.
Full node tables are rebuilt between SpMMs with AllGather (3 per pass).
"""
import os
import numpy as np

N, E, D, K, L = 50000, 800000, 64, 3, 2
NCORES = 8
NSH = N // NCORES              # 6250 dst rows per core
WIN = 32                       # dst rows per window
NWIN = 208                     # windows per core -> 6656 padded cols
NCOLS = NWIN * WIN
CWIN = 8                       # windows per PSUM chunk
NCH = NWIN // CWIN             # 26 chunks
CCOLS = CWIN * WIN             # 256 cols per chunk
TPW = 3                        # tiles per window per table half
ROWS_PAD = NCOLS               # padded rows per core block in the table
TBL_ROWS = NCORES * ROWS_PAD   # 53248
HALF = TBL_ROWS // 2           # 26624 (< 32768: int16-reachable)
NTILE_CH = CWIN * 2 * TPW      # 48 mask tiles per chunk
SLOTS_CH = CWIN * 2 * TPW * 128  # 6144 slots per chunk
SLOTS_TOT = NCH * SLOTS_CH     # 159744
NTILE_TOT = SLOTS_TOT // 128   # 1248
GIDX = 1024                    # rows per dma_gather call
GPC = SLOTS_CH // 2 // GIDX    # 3 gathers per half per chunk

_CACHE = {}


# ---------------------------------------------------------------------------
# Workaround for this walrus build: any instruction carrying >1 sync wait is
# rejected ("Too many sync wait commands"). Hoist extras onto 1-wait NoOps on
# the same engine (per-engine program order preserves semantics).
_ws_counter = [0]


def _split_multiwaits(nc):
    import concourse.mybir as mybir
    n_split = 0
    for fn in nc.m.functions:
        for bb in fn.blocks:
            new_list = []
            changed = False
            for inst in bb.instructions:
                si = inst.sync_info
                waits = list(si.on_wait) if si is not None else []
                if len(waits) > 1:
                    changed = True
                    for w in waits[:-1]:
                        _ws_counter[0] += 1
                        nop = mybir.InstNoOp(
                            name=f"waitsplit-{_ws_counter[0]}",
                            ins=[], outs=[],
                            sync_info=mybir.SyncInfo(on_wait=[w], on_update=[]),
                        )
                        nop.engine = inst.engine
                        nc.register_instruction(nop, overwrite=True)
                        new_list.append(nop)
                        n_split += 1
                    si.on_wait = waits[-1:]
                new_list.append(inst)
            if changed:
                bb.instructions[:] = new_list
    return n_split


def _finalize_with_split(nc):
    import concourse.bass as _bass
    nc.compile()           # Bacc passes (incl. library-load insertion)
    _split_multiwaits(nc)  # after replace_nops_with_events, before freeze
    _bass.Bass.finalize(nc)


def _build_runner(nc, n_cores):
    """SPMD runner over the axon PJRT backend (keeps the jitted executable
    and device-resident inputs so repeat calls can be timed)."""
    import jax
    from jax.sharding import Mesh, PartitionSpec
    from jax.experimental.shard_map import shard_map
    import concourse.mybir as mybir
    from concourse.bass2jax import (
        _bass_exec_p, install_neuronx_cc_hook, partition_id_tensor)

    install_neuronx_cc_hook()
    partition_name = nc.partition_id_tensor.name if nc.partition_id_tensor else None

    in_names, out_names, out_avals, zero_outs = [], [], [], []
    for alloc in nc.m.functions[0].allocations:
        if not isinstance(alloc, mybir.MemoryLocationSet):
            continue
        name = alloc.memorylocations[0].name
        if alloc.kind == "ExternalInput":
            if name != partition_name:
                in_names.append(name)
        elif alloc.kind == "ExternalOutput":
            shape = tuple(alloc.tensor_shape)
            dtype = mybir.dt.np(alloc.dtype)
            out_names.append(name)
            out_avals.append(jax.core.ShapedArray(shape, dtype))
            zero_outs.append(np.zeros(shape, dtype))
    n_params = len(in_names)
    all_in_names = list(in_names) + list(out_names)
    if partition_name is not None:
        all_in_names.append(partition_name)

    def _body(*args):
        operands = list(args)
        if partition_name is not None:
            operands.append(partition_id_tensor())
        outs = _bass_exec_p.bind(
            *operands,
            out_avals=tuple(out_avals),
            in_names=tuple(all_in_names),
            out_names=tuple(out_names),
            lowering_input_output_aliases=(),
            sim_require_finite=True,
            sim_require_nnan=True,
            nc=nc,
        )
        return tuple(outs)

    devices = jax.devices()[:n_cores]
    mesh = Mesh(np.asarray(devices), ("core",))
    in_specs = (PartitionSpec("core"),) * (n_params + len(out_names))
    out_specs = (PartitionSpec("core"),) * len(out_names)
    sharded = jax.jit(
        shard_map(_body, mesh=mesh, in_specs=in_specs, out_specs=out_specs,
                  check_rep=False),
        keep_unused=True,
    )

    def run(in_maps, iters=1):
        import time as _time
        per_core = [[np.asarray(m[name]) for name in in_names] for m in in_maps]
        concat_in = [
            np.concatenate([per_core[c][i] for c in range(n_cores)], axis=0)
            for i in range(n_params)
        ]
        concat_zeros = [
            np.zeros((n_cores * z.shape[0], *z.shape[1:]), z.dtype)
            for z in zero_outs
        ]
        sharding = jax.sharding.NamedSharding(mesh, PartitionSpec("core"))
        dev_in = [jax.device_put(a, sharding) for a in concat_in + concat_zeros]
        out = sharded(*dev_in)
        jax.block_until_ready(out)
        times = []
        for _ in range(iters):
            t0 = _time.perf_counter()
            out = sharded(*dev_in)
            jax.block_until_ready(out)
            times.append(_time.perf_counter() - t0)
        results = [
            {name: np.asarray(out[i]).reshape(n_cores, *out_avals[i].shape)[c]
             for i, name in enumerate(out_names)}
            for c in range(n_cores)
        ]
        return results, times

    return run


def _host_prep(features, src, dst, W, b, pw, pb):
    src = np.asarray(src).astype(np.int64)
    dst = np.asarray(dst).astype(np.int64)
    features = np.asarray(features, dtype=np.float32)
    W = np.asarray(W, dtype=np.float32)
    b = np.asarray(b, dtype=np.float32)
    pw = np.asarray(pw, dtype=np.float32).reshape(D, 1)
    pb = np.asarray(pb, dtype=np.float32).reshape(1)

    deg = np.bincount(dst, minlength=N).astype(np.float32)
    norm = np.clip(deg, 1.0, None) ** -0.5
    wedge = (norm[src] * norm[dst]).astype(np.float32)

    Wflat = np.zeros((D, L * 3 * D), dtype=np.float32)
    for l in range(L):
        for t, Wt in enumerate((W[l, 0] - W[l, 2], -W[l, 1], 2.0 * W[l, 2])):
            Wflat[:, (l * 3 + t) * D:(l * 3 + t + 1) * D] = Wt

    feat_pad = np.zeros((TBL_ROWS, D), dtype=np.float32)
    for j in range(NCORES):
        feat_pad[ROWS_PAD * j:ROWS_PAD * j + NSH] = features[NSH * j:NSH * (j + 1)]

    src_pad = (src // NSH) * ROWS_PAD + (src % NSH)
    core_of = dst // NSH
    dl_all = dst - core_of * NSH

    in_maps = []
    for i in range(NCORES):
        sel = core_of == i
        e_src = src_pad[sel]
        e_dl = dl_all[sel]
        e_w = wedge[sel]
        win = e_dl // WIN
        col = (e_dl % WIN).astype(np.float32)
        half = (e_src >= HALF).astype(np.int64)

        # order edges by (window, half); compute slot within (window, half)
        key = win * 2 + half
        order = np.argsort(key, kind="stable")
        ks = key[order]
        # rank within group
        grp_start = np.searchsorted(ks, np.arange(NWIN * 2), side="left")
        grp_cnt = np.diff(np.append(grp_start, ks.size))
        if grp_cnt.max() > TPW * 128:
            raise RuntimeError(f"window overflow core {i}: {grp_cnt.max()}")
        rank = np.arange(ks.size) - grp_start[ks]

        ww = win[order]
        hh = half[order]
        c_ = ww // CWIN
        wl = ww % CWIN
        slot = (c_ * SLOTS_CH + hh * (SLOTS_CH // 2) + wl * (TPW * 128) + rank)

        idx_slots = np.zeros(SLOTS_TOT, dtype=np.int16)
        col_slots = np.zeros(SLOTS_TOT, dtype=np.float32)
        w_slots = np.zeros(SLOTS_TOT, dtype=np.float32)
        idx_slots[slot] = (e_src[order] - hh * HALF).astype(np.int16)
        col_slots_tmp = col[order]
        w_slots_tmp = e_w[order]

        # mask tile layout: tile index mt within chunk = wl*(2*TPW) + hh*TPW + rank//128
        tj = rank // 128
        part = rank % 128
        mt = c_ * NTILE_CH + wl * (2 * TPW) + hh * TPW + tj
        col_slots[mt * 128 + part] = col_slots_tmp
        w_slots[mt * 128 + part] = w_slots_tmp

        # gather index wrap: gather g slot s -> idx_arr[s%16, g*64 + s//16]
        wrapped = idx_slots.reshape(SLOTS_TOT // GIDX, 64, 16).transpose(0, 2, 1)
        idx_arr16 = wrapped.reshape(SLOTS_TOT // GIDX, 16, 64)
        idx_arr = np.zeros((16, SLOTS_TOT // 16), dtype=np.int16)
        for g in range(SLOTS_TOT // GIDX):
            idx_arr[:, g * 64:(g + 1) * 64] = idx_arr16[g]
        idx_arr = np.tile(idx_arr, (8, 1))

        dcol = col_slots.reshape(NTILE_TOT, 128).T.copy()
        wval = w_slots.reshape(NTILE_TOT, 128).T.copy()

        f0T = np.zeros((D, NCOLS), dtype=np.float32)
        f0T[:, :NSH] = features[NSH * i:NSH * (i + 1)].T

        iota = np.tile(np.arange(WIN, dtype=np.float32)[None, :], (128, 1))

        in_maps.append({
            "feat_pad": feat_pad,
            "f0T": f0T,
            "idx_all": idx_arr,
            "dcol": dcol,
            "wval": wval,
            "iota": iota,
            "Wflat": Wflat,
            "bvec": b.T.copy(),
            "pwv": pw,
            "pbv": pb.reshape(1, 1),
        })
    return in_maps


def _build_nc(repeat=1):
    import concourse.bacc as bacc
    import concourse.mybir as mybir
    import concourse.tile as tile
    from concourse.masks import make_identity
    f32 = mybir.dt.float32

    nc = bacc.Bacc("TRN2", num_swdge_queues=4)
    feat_pad = nc.declare_dram_parameter("feat_pad", [TBL_ROWS, D], f32, isOutput=False)
    f0T_in = nc.declare_dram_parameter("f0T", [D, NCOLS], f32, isOutput=False)
    idx_in = nc.declare_dram_parameter("idx_all", [128, SLOTS_TOT // 16], mybir.dt.int16, isOutput=False)
    dcol_in = nc.declare_dram_parameter("dcol", [128, NTILE_TOT], f32, isOutput=False)
    wval_in = nc.declare_dram_parameter("wval", [128, NTILE_TOT], f32, isOutput=False)
    iota_in = nc.declare_dram_parameter("iota", [128, WIN], f32, isOutput=False)
    W_in = nc.declare_dram_parameter("Wflat", [D, L * 3 * D], f32, isOutput=False)
    b_in = nc.declare_dram_parameter("bvec", [D, L], f32, isOutput=False)
    pw_in = nc.declare_dram_parameter("pwv", [D, 1], f32, isOutput=False)
    pb_in = nc.declare_dram_parameter("pbv", [1, 1], f32, isOutput=False)
    y_out = nc.declare_dram_parameter("y", [NSH, 1], f32, isOutput=True)

    ag = {}
    for nm in ("u1", "h1", "u1b"):
        ag[nm] = (
            nc.dram_tensor(f"agin_{nm}", [ROWS_PAD, D], f32),
            nc.dram_tensor(f"agout_{nm}", [TBL_ROWS, D], f32, addr_space="Shared"),
        )

    with tile.TileContext(nc) as tc:
        with (
            tc.tile_pool(name="const", bufs=1) as cp,
            tc.tile_pool(name="idxp", bufs=2) as ip,
            tc.tile_pool(name="gbuf", bufs=2) as gp,
            tc.tile_pool(name="mbuf", bufs=2) as mp,
            tc.tile_pool(name="sT", bufs=1) as sp,
            tc.tile_pool(name="rows", bufs=1) as rp,
            tc.tile_pool(name="spsum", bufs=2, space="PSUM") as pp,
            tc.tile_pool(name="opsum", bufs=2, space="PSUM") as tp,
        ):
            dcol = cp.tile([128, NTILE_TOT], f32)
            nc.sync.dma_start(out=dcol[:], in_=dcol_in[:])
            wval = cp.tile([128, NTILE_TOT], f32)
            nc.sync.dma_start(out=wval[:], in_=wval_in[:])
            iota = cp.tile([128, WIN], f32)
            nc.sync.dma_start(out=iota[:], in_=iota_in[:])
            wfl = cp.tile([D, L * 3 * D], f32)
            nc.sync.dma_start(out=wfl[:], in_=W_in[:])
            bv = cp.tile([D, L], f32)
            nc.sync.dma_start(out=bv[:], in_=b_in[:])
            pwv = cp.tile([D, 1], f32)
            nc.sync.dma_start(out=pwv[:], in_=pw_in[:])
            pbv = cp.tile([1, 1], f32)
            nc.sync.dma_start(out=pbv[:], in_=pb_in[:])
            f0T = cp.tile([D, NCOLS], f32)
            nc.sync.dma_start(out=f0T[:], in_=f0T_in[:])
            ident = cp.tile([128, 128], f32)
            make_identity(nc, ident[:])

            u1T = sp.tile([D, NCOLS], f32, tag="u1T")
            h1T = sp.tile([D, NCOLS], f32, tag="h1T")
            yrow = sp.tile([1, NCOLS], f32, tag="yrow")

            gq = [0]

            def spmm_chunk(table, c, tag):
                """Gathers + mask gen + PE reduce for chunk c. Returns psum
                tile [64, CCOLS] (caller evacuates / consumes)."""
                idxc = ip.tile([128, SLOTS_CH // 16], mybir.dt.int16, tag="idxc",
                               name=f"idxc_{tag}_{c}")
                nc.sync.dma_start(
                    out=idxc[:],
                    in_=idx_in[:, c * (SLOTS_CH // 16):(c + 1) * (SLOTS_CH // 16)])
                glo = gp.tile([128, CWIN * TPW, D], f32, tag="glo", name=f"glo_{tag}_{c}")
                ghi = gp.tile([128, CWIN * TPW, D], f32, tag="ghi", name=f"ghi_{tag}_{c}")
                for h, gbuf in ((0, glo), (1, ghi)):
                    tab = table[h * HALF:(h + 1) * HALF, :]
                    for g in range(GPC):
                        off = (h * (SLOTS_CH // 2) + g * GIDX) // 16
                        nc.gpsimd.dma_gather(
                            gbuf[:, g * (GIDX // 128):(g + 1) * (GIDX // 128), :],
                            tab,
                            idxc[:, off:off + GIDX // 16],
                            GIDX, GIDX, D,
                            queue_num=gq[0] % 4,
                        )
                        gq[0] += 1
                mask = mp.tile([128, NTILE_CH * WIN], f32, tag="mask",
                               name=f"mask_{tag}_{c}")
                m3 = mask[:].rearrange("p (t o) -> p t o", o=WIN)
                i3 = iota[:].rearrange("p (o t) -> p o t", o=1).to_broadcast(
                    [128, NTILE_CH, WIN])
                d3 = dcol[:, c * NTILE_CH:(c + 1) * NTILE_CH].rearrange(
                    "p (t o) -> p t o", o=1).to_broadcast([128, NTILE_CH, WIN])
                w3 = wval[:, c * NTILE_CH:(c + 1) * NTILE_CH].rearrange(
                    "p (t o) -> p t o", o=1).to_broadcast([128, NTILE_CH, WIN])
                nc.vector.tensor_tensor(out=m3, in0=i3, in1=d3,
                                        op=mybir.AluOpType.is_equal)
                nc.vector.tensor_tensor(out=m3, in0=m3, in1=w3,
                                        op=mybir.AluOpType.mult)
                ps = pp.tile([64, CCOLS], f32, tag="spsum", name=f"ps_{tag}_{c}")
                for w in range(CWIN):
                    for j in range(2 * TPW):
                        lhsT = (glo if j < TPW else ghi)[:, TPW * w + (j % TPW), :]
                        mt = w * (2 * TPW) + j
                        nc.tensor.matmul(
                            ps[:, WIN * w:WIN * (w + 1)],
                            lhsT,
                            mask[:, mt * WIN:(mt + 1) * WIN],
                            start=(j == 0), stop=(j == 2 * TPW - 1),
                        )
                return ps

            def spmm(table, out_sT, tag):
                for c in range(NCH):
                    ps = spmm_chunk(table, c, tag)
                    nc.vector.tensor_copy(
                        out=out_sT[:, c * CCOLS:(c + 1) * CCOLS], in_=ps[:])

            def spmm_fused_dense(table, l, u0T, u1T_, outT, tag):
                """SpMM for U2 fused with the dense layer + (layer 2) head."""
                for c in range(NCH):
                    ps = spmm_chunk(table, c, tag)
                    u2c = mp.tile([64, CCOLS], f32, tag="u2c", name=f"u2c_{tag}_{c}")
                    nc.vector.tensor_copy(out=u2c[:], in_=ps[:])
                    dp = tp.tile([64, CCOLS], f32, tag="dpsum", name=f"dp_{tag}_{c}")
                    for t, uT in enumerate((u0T, u1T_, u2c)):
                        rhs = uT[:] if t == 2 else uT[:, c * CCOLS:(c + 1) * CCOLS]
                        nc.tensor.matmul(
                            dp[:],
                            wfl[:, (l * 3 + t) * D:(l * 3 + t + 1) * D],
                            rhs,
                            start=(t == 0), stop=(t == 2),
                        )
                    nc.scalar.activation(
                        out=outT[:, c * CCOLS:(c + 1) * CCOLS],
                        in_=dp[:],
                        func=mybir.ActivationFunctionType.Relu,
                        bias=bv[:, l:l + 1],
                        scale=1.0,
                    )
                    if l == L - 1:
                        hp = tp.tile([1, CCOLS], f32, tag="hpsum", name=f"hp_{c}")
                        nc.tensor.matmul(
                            hp[:], pwv[:],
                            outT[:, c * CCOLS:(c + 1) * CCOLS],
                            start=True, stop=True)
                        nc.vector.tensor_scalar(
                            out=yrow[:1, c * CCOLS:(c + 1) * CCOLS],
                            in0=hp[:], scalar1=pbv[:1, :1], scalar2=None,
                            op0=mybir.AluOpType.add)

            def transpose_out(sT, agin, tag):
                rows = rp.tile([128, NCOLS // 128, D], f32, tag="rows",
                               name=f"rows_{tag}")
                for k in range(NCOLS // 128):
                    tps = tp.tile([128, D], f32, tag="tpsum", name=f"tps_{tag}_{k}")
                    nc.tensor.transpose(tps[:], sT[:, k * 128:(k + 1) * 128],
                                        ident[:64, :64])
                    nc.vector.tensor_copy(out=rows[:, k, :], in_=tps[:])
                nc.sync.dma_start(
                    out=agin.ap().rearrange("(k p) d -> p k d", p=128), in_=rows[:])

            def allgather(nm):
                agin, agout = ag[nm]
                nc.gpsimd.collective_compute(
                    "AllGather",
                    mybir.AluOpType.bypass,
                    ins=[agin.ap().opt()],
                    outs=[agout.ap().opt()],
                    replica_groups=[list(range(NCORES))],
                )

            h2T = sp.tile([D, NCOLS], f32, tag="h2T")
            for r in range(repeat):
                # ---- layer 1 ----
                spmm(feat_pad, u1T, f"r{r}s1")
                transpose_out(u1T, ag["u1"][0], f"r{r}t1")
                allgather("u1")
                # h1T <- relu(f0 Wa + u1 Wb + u2 Wc + b0), u2 fused from SpMM2
                spmm_fused_dense(ag["u1"][1], 0, f0T, u1T, h1T, f"r{r}s2")
                transpose_out(h1T, ag["h1"][0], f"r{r}t2")
                allgather("h1")
                # ---- layer 2 ----
                spmm(ag["h1"][1], u1T, f"r{r}s3")
                transpose_out(u1T, ag["u1b"][0], f"r{r}t3")
                allgather("u1b")
                spmm_fused_dense(ag["u1b"][1], 1, h1T, u1T, h2T, f"r{r}s4")
            nc.sync.dma_start(out=y_out[:, :], in_=yrow[:1, :NSH])

    _finalize_with_split(nc)
    return nc


def _get_runner():
    if "runner" in _CACHE:
        return _CACHE["runner"]
    nc = _build_nc()
    _CACHE["runner"] = _build_runner(nc, NCORES)
    return _CACHE["runner"]


def kernel(features, src, dst, W, b, pw, pb):
    in_maps = _host_prep(features, src, dst, W, b, pw, pb)
    run = _get_runner()
    results, times = run(in_maps, iters=1)
    _CACHE["last_times"] = times
    y = np.concatenate([results[i]["y"] for i in range(NCORES)], axis=0)
    return y.astype(np.float32)
